# revision 10
# baseline (speedup 1.0000x reference)
"""Trainium2 Bass kernel for nn_DownwardPropagation.

Math (per batch row b, channel c, layers l=1..L):
    fd_l = fd_{l-1} * td_l                       (direct downward flux)
    ff_l = (tdf_l + tmf_l) * ff_{l-1} + fd_{l-1} * tmd_l
    up_l = fd_{l-1} * rbd_l + ff_{l-1} * rbf_l
    ab_l = fd_{l-1} * atd_l + ff_{l-1} * atf_l
Outputs are channel sums: fd_sums[B, L+1], ff_sums[B, L+1],
up_sums[B, L+1] (up_0 = sum_c fd_0 * r_multi_direct), ab_sums[B, L].

Kernel layout: batch sharded 8 ways (2048 rows/core), each core processes
16 chunks of 128 rows (partition dim). Per chunk, layers are processed in
blocks of LB. The layer recurrences run as ONE tensor_tensor_scan per
carry per block over a c-major "grid" layout [C, G=LB+1]: slot (c,0) is a
boundary element with data0=0 (kills the cross-channel state leak) and
data1=carry-in (injects the per-channel initial value); slots (c,1..LB)
hold the per-layer coefficients. The scan output grid then holds
fd_{l0..l0+LB} per channel directly.
"""

import numpy as np

_B, _L, _C = 16384, 60, 48
_NCORES = 8
_P = 128
_LB = 20

PROPS = [
    "t_direct", "t_diffuse", "t_multi_direct", "t_multi_diffuse",
    "r_bottom_multi_direct", "r_bottom_multi_diffuse",
    "a_top_multi_direct", "a_top_multi_diffuse",
]
FLUX = ["r_multi_direct", "flux_down_above_direct", "flux_down_above_diffuse"]


def build_nc(n_rows=_B // _NCORES, L=_L, C=_C, LB=_LB, n_cores=_NCORES):
    import concourse.bacc as bacc
    import concourse.mybir as mybir
    from concourse.tile import TileContext

    f32 = mybir.dt.float32
    AL = mybir.AluOpType
    AX = mybir.AxisListType
    P = _P
    assert n_rows % P == 0 and L % LB == 0
    n_chunks = n_rows // P
    NBLK = L // LB
    G = LB + 1

    nc = bacc.Bacc("TRN2", target_bir_lowering=False, debug=False,
                   num_devices=n_cores)

    d_in = {n: nc.dram_tensor(n, [n_rows, L, C], f32, kind="ExternalInput").ap()
            for n in PROPS}
    d_fx = {n: nc.dram_tensor(n, [n_rows, C], f32, kind="ExternalInput").ap()
            for n in FLUX}
    d_fds = nc.dram_tensor("out_fds", [n_rows, L + 1], f32, kind="ExternalOutput").ap()
    d_ffs = nc.dram_tensor("out_ffs", [n_rows, L + 1], f32, kind="ExternalOutput").ap()
    d_ups = nc.dram_tensor("out_ups", [n_rows, L + 1], f32, kind="ExternalOutput").ap()
    d_abs = nc.dram_tensor("out_abs", [n_rows, L], f32, kind="ExternalOutput").ap()

    with TileContext(nc) as tc:
        with (
            tc.tile_pool(name="inp", bufs=2) as pool_in,
            tc.tile_pool(name="grids", bufs=2) as pool_grid,
            tc.tile_pool(name="seq", bufs=3) as pool_seq,
            tc.tile_pool(name="prod", bufs=3) as pool_prod,
            tc.tile_pool(name="outs", bufs=2) as pool_out,
            tc.tile_pool(name="small", bufs=2) as pool_small,
            tc.tile_pool(name="persist", bufs=1) as pool_persist,
        ):
            # Persistent injection grids, alternating per chunk parity.
            # Zeroed once; only their boundary column (tdc/ac: never) is
            # rewritten afterwards, so the zero interior persists.
            fd1 = [pool_persist.tile([P, C * G], f32, name=f"fd1_{i}", tag=f"fd1_{i}") for i in range(2)]
            tdc = [pool_persist.tile([P, C * G], f32, name=f"tdc_{i}", tag=f"tdc_{i}") for i in range(2)]
            ac = [pool_persist.tile([P, C * G], f32, name=f"ac_{i}", tag=f"ac_{i}") for i in range(2)]
            for t in (*fd1, *tdc, *ac):
                nc.vector.memset(t[:], 0.0)

            def co3(t):  # c-order view of an l-major [P, LB*C] tile -> [P, C, LB]
                return t.rearrange("p (l c) -> p l c", c=C).transpose([0, 2, 1])

            def lm3(t):  # l-major view [P, LB, C], contiguous
                return t.rearrange("p (l c) -> p l c", c=C)

            def chunk_prologue(ch):
                r0 = ch * P
                st = {"ch": ch, "r0": r0}
                st["fd1_3"] = fd1[ch % 2].rearrange("p (c g) -> p c g", g=G)
                st["tdc_g"], st["ac_g"], st["fd1_g"] = \
                    tdc[ch % 2], ac[ch % 2], fd1[ch % 2]
                st["tdc_3"] = tdc[ch % 2].rearrange("p (c g) -> p c g", g=G)
                st["ac_3"] = ac[ch % 2].rearrange("p (c g) -> p c g", g=G)

                fd0 = pool_small.tile([P, C], f32, tag="fd0")
                ff0 = pool_small.tile([P, C], f32, tag="ff0")
                rmd = pool_small.tile([P, C], f32, tag="rmd")
                nc.sync.dma_start(out=fd0[:], in_=d_fx["flux_down_above_direct"][r0:r0 + P])
                nc.sync.dma_start(out=ff0[:], in_=d_fx["flux_down_above_diffuse"][r0:r0 + P])
                nc.sync.dma_start(out=rmd[:], in_=d_fx["r_multi_direct"][r0:r0 + P])
                st["fd0"], st["ff0"] = fd0, ff0

                st["t_fds"] = pool_out.tile([P, L + 1], f32, name="t_fds", tag="o_fds")
                st["t_ffs"] = pool_out.tile([P, L + 1], f32, name="t_ffs", tag="o_ffs")
                st["t_ups"] = pool_out.tile([P, L + 1], f32, name="t_ups", tag="o_ups")
                st["t_abs"] = pool_out.tile([P, L], f32, name="t_abs", tag="o_abs")

                # up_0 = sum_c fd0 * rmd  (accum_out of a fused mult)
                trash = pool_small.tile([P, C], f32, tag="trash")
                nc.vector.scalar_tensor_tensor(
                    out=trash[:], in0=fd0[:], scalar=1.0, in1=rmd[:],
                    op0=AL.mult, op1=AL.mult, accum_out=st["t_ups"][:, 0:1])
                return st

            def chunk_block(st, bl):
                r0, l0 = st["r0"], bl * LB
                fd1_3, tdc_3, ac_3 = st["fd1_3"], st["tdc_3"], st["ac_3"]
                tin = {}
                for name in PROPS:
                    t = pool_in.tile([P, LB * C], f32, name=f"in_{name}",
                                     tag=f"in_{name}",
                                     bufs=3 if name in PROPS[:4] else 2)
                    nc.sync.dma_start(
                        out=t[:],
                        in_=d_in[name][r0:r0 + P, l0:l0 + LB].rearrange(
                            "p l c -> p (l c)"))
                    tin[name] = t

                # boundary injection for the fd scan (data1 col 0)
                if bl == 0:
                    nc.scalar.copy(out=fd1_3[:, :, 0:1],
                                   in_=st["fd0"][:].unsqueeze(2))
                else:
                    nc.scalar.copy(out=fd1_3[:, :, 0:1],
                                   in_=st["prev_fd_c3"][:, :, LB:LB + 1])
                # td -> c-major grid (cols 1..G); col 0 stays 0
                nc.scalar.copy(out=tdc_3[:, :, 1:G], in_=co3(tin["t_direct"]))
                # a = tdf + tmf: contiguous add on DVE, then ACT moves it
                # into the c-major grid (strided APs are ~2x slower on
                # DVE but cheap to absorb on the ACT engine)
                a_lm = pool_grid.tile([P, LB * C], f32, tag="a_lm")
                nc.vector.tensor_add(out=a_lm[:], in0=tin["t_diffuse"][:],
                                     in1=tin["t_multi_diffuse"][:])
                nc.scalar.copy(out=ac_3[:, :, 1:G], in_=co3(a_lm))
                # tmd -> c-major grid so the b-mult below is all-contiguous
                tmd_c = pool_grid.tile([P, C * G], f32, tag="tmd_c")
                tmdc_3 = tmd_c.rearrange("p (c g) -> p c g", g=G)
                nc.scalar.copy(out=tmdc_3[:, :, 1:G],
                               in_=co3(tin["t_multi_direct"]))

                fd_c = pool_seq.tile([P, C * G], f32, tag="fd_c")
                nc.vector.tensor_tensor_scan(
                    out=fd_c[:], data0=st["tdc_g"][:], data1=st["fd1_g"][:],
                    initial=0.0, op0=AL.mult, op1=AL.add)
                fd_c3 = fd_c.rearrange("p (c g) -> p c g", g=G)

                # b grid: col0 = ff carry, cols 1.. = fd_{l-1} * tmd_l
                b_buf = pool_grid.tile([P, C * G], f32, tag="b_buf", bufs=1)
                b_3 = b_buf.rearrange("p (c g) -> p c g", g=G)
                if bl == 0:
                    nc.scalar.copy(out=b_3[:, :, 0:1],
                                   in_=st["ff0"][:].unsqueeze(2))
                else:
                    nc.scalar.copy(out=b_3[:, :, 0:1],
                                   in_=st["prev_ff_c3"][:, :, LB:LB + 1])
                nc.gpsimd.tensor_mul(out=b_3[:, :, 1:G],
                                     in0=fd_c3[:, :, 0:LB],
                                     in1=tmdc_3[:, :, 1:G])

                ff_c = pool_seq.tile([P, C * G], f32, tag="ff_c")
                nc.vector.tensor_tensor_scan(
                    out=ff_c[:], data0=st["ac_g"][:], data1=b_buf[:],
                    initial=0.0, op0=AL.mult, op1=AL.add)
                ff_c3 = ff_c.rearrange("p (c g) -> p c g", g=G)

                # shifted (carry-in) sequences, l-major views [P, LB, C]
                fd_sh = fd_c3.transpose([0, 2, 1])[:, 0:LB, :]
                ff_sh = ff_c3.transpose([0, 2, 1])[:, 0:LB, :]

                # products (GPSIMD)
                u_pair = pool_prod.tile([P, 2 * LB * C], f32, tag="u_pair", bufs=2)
                a_pair = pool_prod.tile([P, 2 * LB * C], f32, tag="a_pair", bufs=2)
                u0 = u_pair[:, 0:LB * C].rearrange("p (l c) -> p l c", c=C)
                u1 = u_pair[:, LB * C:].rearrange("p (l c) -> p l c", c=C)
                a0 = a_pair[:, 0:LB * C].rearrange("p (l c) -> p l c", c=C)
                a1 = a_pair[:, LB * C:].rearrange("p (l c) -> p l c", c=C)
                nc.gpsimd.tensor_mul(out=u0, in0=fd_sh,
                                     in1=lm3(tin["r_bottom_multi_direct"]))
                nc.gpsimd.tensor_mul(out=u1, in0=ff_sh,
                                     in1=lm3(tin["r_bottom_multi_diffuse"]))
                nc.gpsimd.tensor_mul(out=a0, in0=fd_sh,
                                     in1=lm3(tin["a_top_multi_direct"]))
                nc.gpsimd.tensor_mul(out=a1, in0=ff_sh,
                                     in1=lm3(tin["a_top_multi_diffuse"]))

                # fused pair reduction over (pair, c) -> per-layer sums
                u_red = u_pair.rearrange("p (t l c) -> p t l c", t=2, c=C
                                         ).transpose([0, 2, 1, 3])
                a_red = a_pair.rearrange("p (t l c) -> p t l c", t=2, c=C
                                         ).transpose([0, 2, 1, 3])
                nc.vector.tensor_reduce(out=st["t_ups"][:, l0 + 1:l0 + LB + 1],
                                        in_=u_red, axis=AX.XY, op=AL.add)
                nc.vector.tensor_reduce(out=st["t_abs"][:, l0:l0 + LB],
                                        in_=a_red, axis=AX.XY, op=AL.add)

                # per-level channel sums of the carries: ACT transposes
                # the carry-in slots to l-major lanes so the DVE reduce
                # gets a contiguous innermost axis (2x faster than
                # reducing the c-major grid directly). Covers levels
                # l0..l0+LB-1; the final level L is a small fixup below.
                fd_lane = pool_prod.tile([P, LB * C], f32, tag="fd_lane", bufs=2)
                ff_lane = pool_prod.tile([P, LB * C], f32, tag="ff_lane", bufs=2)
                nc.scalar.copy(out=lm3(fd_lane), in_=fd_sh)
                nc.scalar.copy(out=lm3(ff_lane), in_=ff_sh)
                nc.vector.tensor_reduce(out=st["t_fds"][:, l0:l0 + LB],
                                        in_=lm3(fd_lane), axis=AX.X,
                                        op=AL.add)
                nc.vector.tensor_reduce(out=st["t_ffs"][:, l0:l0 + LB],
                                        in_=lm3(ff_lane), axis=AX.X,
                                        op=AL.add)
                if bl == NBLK - 1:
                    nc.vector.tensor_reduce(
                        out=st["t_fds"][:, L:L + 1],
                        in_=fd_c3.transpose([0, 2, 1])[:, LB:LB + 1, :],
                        axis=AX.X, op=AL.add)
                    nc.vector.tensor_reduce(
                        out=st["t_ffs"][:, L:L + 1],
                        in_=ff_c3.transpose([0, 2, 1])[:, LB:LB + 1, :],
                        axis=AX.X, op=AL.add)

                st["prev_fd_c3"], st["prev_ff_c3"] = fd_c3, ff_c3

            def chunk_epilogue(st):
                r0 = st["r0"]
                nc.sync.dma_start(out=d_fds[r0:r0 + P], in_=st["t_fds"][:])
                nc.sync.dma_start(out=d_ffs[r0:r0 + P], in_=st["t_ffs"][:])
                nc.sync.dma_start(out=d_ups[r0:r0 + P], in_=st["t_ups"][:])
                nc.sync.dma_start(out=d_abs[r0:r0 + P], in_=st["t_abs"][:])

            # Software-pipeline two independent chunks at block granularity:
            # each engine's FIFO alternates between the two carry chains, so
            # while one chain waits on a scan the engine runs the other
            # chain's ready work instead of head-of-line stalling.
            for ch0 in range(0, n_chunks, 2):
                group = [chunk_prologue(ch0 + s)
                         for s in range(min(2, n_chunks - ch0))]
                for bl in range(NBLK):
                    for st in group:
                        chunk_block(st, bl)
                for st in group:
                    chunk_epilogue(st)

    nc.compile()
    return nc


_NC_CACHE = {}


def _get_nc(key=("full",)):
    if key not in _NC_CACHE:
        _NC_CACHE[key] = build_nc()
    return _NC_CACHE[key]


def kernel(**inputs):
    """Full-problem entry point: shard over 8 cores, run, gather."""
    from concourse.bass_utils import run_bass_kernel_spmd

    nc = _get_nc()
    rows = _B // _NCORES
    in_maps = []
    for core in range(_NCORES):
        sl = slice(core * rows, (core + 1) * rows)
        m = {n: np.ascontiguousarray(np.asarray(inputs[n])[sl], dtype=np.float32)
             for n in PROPS + FLUX}
        in_maps.append(m)

    res = run_bass_kernel_spmd(nc, in_maps, core_ids=list(range(_NCORES)))
    fds = np.concatenate([r["out_fds"] for r in res.results], axis=0)
    ffs = np.concatenate([r["out_ffs"] for r in res.results], axis=0)
    ups = np.concatenate([r["out_ups"] for r in res.results], axis=0)
    abs_ = np.concatenate([r["out_abs"] for r in res.results], axis=0)
    return fds, ffs, ups, abs_


# revision 11
# speedup vs baseline: 1.1917x; 1.1917x over previous
"""Trainium2 Bass kernel for nn_DownwardPropagation.

Math (per batch row b, channel c, layers l=1..L):
    fd_l = fd_{l-1} * td_l                       (direct downward flux)
    ff_l = (tdf_l + tmf_l) * ff_{l-1} + fd_{l-1} * tmd_l
    up_l = fd_{l-1} * rbd_l + ff_{l-1} * rbf_l
    ab_l = fd_{l-1} * atd_l + ff_{l-1} * atf_l
Outputs are channel sums: fd_sums[B, L+1], ff_sums[B, L+1],
up_sums[B, L+1] (up_0 = sum_c fd_0 * r_multi_direct), ab_sums[B, L].

Kernel layout: batch sharded 8 ways (2048 rows/core), each core processes
16 chunks of 128 rows (partition dim). Per chunk, layers are processed in
blocks of LB. The layer recurrences run as ONE tensor_tensor_scan per
carry per block over a c-major "grid" layout [C, G=LB+1]: slot (c,0) is a
boundary element with data0=0 (kills the cross-channel state leak) and
data1=carry-in (injects the per-channel initial value); slots (c,1..LB)
hold the per-layer coefficients. The scan output grid then holds
fd_{l0..l0+LB} per channel directly.
"""

import numpy as np

_B, _L, _C = 16384, 60, 48
_NCORES = 8
_P = 128
_LB = 20

PROPS = [
    "t_direct", "t_diffuse", "t_multi_direct", "t_multi_diffuse",
    "r_bottom_multi_direct", "r_bottom_multi_diffuse",
    "a_top_multi_direct", "a_top_multi_diffuse",
]
FLUX = ["r_multi_direct", "flux_down_above_direct", "flux_down_above_diffuse"]


def build_nc(n_rows=_B // _NCORES, L=_L, C=_C, LB=_LB, n_cores=_NCORES):
    import concourse.bacc as bacc
    import concourse.mybir as mybir
    from concourse.tile import TileContext

    f32 = mybir.dt.float32
    AL = mybir.AluOpType
    AX = mybir.AxisListType
    P = _P
    assert n_rows % P == 0 and L % LB == 0
    n_chunks = n_rows // P
    NBLK = L // LB
    G = LB + 1

    nc = bacc.Bacc("TRN2", target_bir_lowering=False, debug=False,
                   num_devices=n_cores)

    d_in = {n: nc.dram_tensor(n, [n_rows, L, C], f32, kind="ExternalInput").ap()
            for n in PROPS}
    d_fx = {n: nc.dram_tensor(n, [n_rows, C], f32, kind="ExternalInput").ap()
            for n in FLUX}
    d_fds = nc.dram_tensor("out_fds", [n_rows, L + 1], f32, kind="ExternalOutput").ap()
    d_ffs = nc.dram_tensor("out_ffs", [n_rows, L + 1], f32, kind="ExternalOutput").ap()
    d_ups = nc.dram_tensor("out_ups", [n_rows, L + 1], f32, kind="ExternalOutput").ap()
    d_abs = nc.dram_tensor("out_abs", [n_rows, L], f32, kind="ExternalOutput").ap()

    with TileContext(nc) as tc:
        with (
            tc.tile_pool(name="inp", bufs=2) as pool_in,
            tc.tile_pool(name="grids", bufs=2) as pool_grid,
            tc.tile_pool(name="seq", bufs=3) as pool_seq,
            tc.tile_pool(name="prod", bufs=3) as pool_prod,
            tc.tile_pool(name="outs", bufs=2) as pool_out,
            tc.tile_pool(name="small", bufs=2) as pool_small,
            tc.tile_pool(name="persist", bufs=1) as pool_persist,
        ):
            # Persistent injection grids, alternating per chunk parity.
            # Zeroed once; only their boundary column (tdc/ac: never) is
            # rewritten afterwards, so the zero interior persists.
            fd1 = [pool_persist.tile([P, C * G], f32, name=f"fd1_{i}", tag=f"fd1_{i}") for i in range(2)]
            tdc = [pool_persist.tile([P, C * G], f32, name=f"tdc_{i}", tag=f"tdc_{i}") for i in range(2)]
            ac = [pool_persist.tile([P, C * G], f32, name=f"ac_{i}", tag=f"ac_{i}") for i in range(2)]
            for t in (*fd1, *tdc, *ac):
                nc.vector.memset(t[:], 0.0)

            def co3(t):  # c-order view of an l-major [P, LB*C] tile -> [P, C, LB]
                return t.rearrange("p (l c) -> p l c", c=C).transpose([0, 2, 1])

            def lm3(t):  # l-major view [P, LB, C], contiguous
                return t.rearrange("p (l c) -> p l c", c=C)

            def chunk_prologue(ch):
                r0 = ch * P
                st = {"ch": ch, "r0": r0}
                st["fd1_3"] = fd1[ch % 2].rearrange("p (c g) -> p c g", g=G)
                st["tdc_g"], st["ac_g"], st["fd1_g"] = \
                    tdc[ch % 2], ac[ch % 2], fd1[ch % 2]
                st["tdc_3"] = tdc[ch % 2].rearrange("p (c g) -> p c g", g=G)
                st["ac_3"] = ac[ch % 2].rearrange("p (c g) -> p c g", g=G)

                fd0 = pool_small.tile([P, C], f32, tag="fd0")
                ff0 = pool_small.tile([P, C], f32, tag="ff0")
                rmd = pool_small.tile([P, C], f32, tag="rmd")
                nc.sync.dma_start(out=fd0[:], in_=d_fx["flux_down_above_direct"][r0:r0 + P])
                nc.sync.dma_start(out=ff0[:], in_=d_fx["flux_down_above_diffuse"][r0:r0 + P])
                nc.sync.dma_start(out=rmd[:], in_=d_fx["r_multi_direct"][r0:r0 + P])
                st["fd0"], st["ff0"] = fd0, ff0

                st["t_fds"] = pool_out.tile([P, L + 1], f32, name="t_fds", tag="o_fds")
                st["t_ffs"] = pool_out.tile([P, L + 1], f32, name="t_ffs", tag="o_ffs")
                st["t_ups"] = pool_out.tile([P, L + 1], f32, name="t_ups", tag="o_ups")
                st["t_abs"] = pool_out.tile([P, L], f32, name="t_abs", tag="o_abs")

                # up_0 = sum_c fd0 * rmd  (accum_out of a fused mult)
                trash = pool_small.tile([P, C], f32, tag="trash")
                nc.vector.scalar_tensor_tensor(
                    out=trash[:], in0=fd0[:], scalar=1.0, in1=rmd[:],
                    op0=AL.mult, op1=AL.mult, accum_out=st["t_ups"][:, 0:1])
                return st

            def chunk_block(st, bl):
                r0, l0 = st["r0"], bl * LB
                fd1_3, tdc_3, ac_3 = st["fd1_3"], st["tdc_3"], st["ac_3"]
                tin = {}
                for name in PROPS:
                    t = pool_in.tile([P, LB * C], f32, name=f"in_{name}",
                                     tag=f"in_{name}",
                                     bufs=3 if name in PROPS[:4] else 2)
                    nc.sync.dma_start(
                        out=t[:],
                        in_=d_in[name][r0:r0 + P, l0:l0 + LB].rearrange(
                            "p l c -> p (l c)"))
                    tin[name] = t

                # boundary injection for the fd scan (data1 col 0)
                if bl == 0:
                    nc.scalar.copy(out=fd1_3[:, :, 0:1],
                                   in_=st["fd0"][:].unsqueeze(2))
                else:
                    nc.scalar.copy(out=fd1_3[:, :, 0:1],
                                   in_=st["prev_fd_c3"][:, :, LB:LB + 1])
                # td -> c-major grid (cols 1..G); col 0 stays 0
                nc.scalar.copy(out=tdc_3[:, :, 1:G], in_=co3(tin["t_direct"]))
                # a = tdf + tmf: contiguous add on DVE, then ACT moves it
                # into the c-major grid (strided APs are ~2x slower on
                # DVE but cheap to absorb on the ACT engine)
                a_lm = pool_grid.tile([P, LB * C], f32, tag="a_lm")
                nc.vector.tensor_add(out=a_lm[:], in0=tin["t_diffuse"][:],
                                     in1=tin["t_multi_diffuse"][:])
                nc.scalar.copy(out=ac_3[:, :, 1:G], in_=co3(a_lm))
                # tmd -> c-major grid so the b-mult below is all-contiguous
                tmd_c = pool_grid.tile([P, C * G], f32, tag="tmd_c")
                tmdc_3 = tmd_c.rearrange("p (c g) -> p c g", g=G)
                nc.scalar.copy(out=tmdc_3[:, :, 1:G],
                               in_=co3(tin["t_multi_direct"]))

                fd_c = pool_seq.tile([P, C * G], f32, tag="fd_c")
                nc.vector.tensor_tensor_scan(
                    out=fd_c[:], data0=st["tdc_g"][:], data1=st["fd1_g"][:],
                    initial=0.0, op0=AL.mult, op1=AL.add)
                fd_c3 = fd_c.rearrange("p (c g) -> p c g", g=G)

                # b grid: col0 = ff carry, cols 1.. = fd_{l-1} * tmd_l
                b_buf = pool_grid.tile([P, C * G], f32, tag="b_buf", bufs=2)
                b_3 = b_buf.rearrange("p (c g) -> p c g", g=G)
                if bl == 0:
                    nc.scalar.copy(out=b_3[:, :, 0:1],
                                   in_=st["ff0"][:].unsqueeze(2))
                else:
                    nc.scalar.copy(out=b_3[:, :, 0:1],
                                   in_=st["prev_ff_c3"][:, :, LB:LB + 1])
                nc.gpsimd.tensor_mul(out=b_3[:, :, 1:G],
                                     in0=fd_c3[:, :, 0:LB],
                                     in1=tmdc_3[:, :, 1:G])

                ff_c = pool_seq.tile([P, C * G], f32, tag="ff_c")
                nc.vector.tensor_tensor_scan(
                    out=ff_c[:], data0=st["ac_g"][:], data1=b_buf[:],
                    initial=0.0, op0=AL.mult, op1=AL.add)
                ff_c3 = ff_c.rearrange("p (c g) -> p c g", g=G)

                # shifted (carry-in) sequences, l-major views [P, LB, C]
                fd_sh = fd_c3.transpose([0, 2, 1])[:, 0:LB, :]
                ff_sh = ff_c3.transpose([0, 2, 1])[:, 0:LB, :]

                # products (GPSIMD)
                u_pair = pool_prod.tile([P, 2 * LB * C], f32, tag="u_pair", bufs=2)
                a_pair = pool_prod.tile([P, 2 * LB * C], f32, tag="a_pair", bufs=2)
                u0 = u_pair[:, 0:LB * C].rearrange("p (l c) -> p l c", c=C)
                u1 = u_pair[:, LB * C:].rearrange("p (l c) -> p l c", c=C)
                a0 = a_pair[:, 0:LB * C].rearrange("p (l c) -> p l c", c=C)
                a1 = a_pair[:, LB * C:].rearrange("p (l c) -> p l c", c=C)
                nc.gpsimd.tensor_mul(out=u0, in0=fd_sh,
                                     in1=lm3(tin["r_bottom_multi_direct"]))
                nc.gpsimd.tensor_mul(out=u1, in0=ff_sh,
                                     in1=lm3(tin["r_bottom_multi_diffuse"]))
                nc.gpsimd.tensor_mul(out=a0, in0=fd_sh,
                                     in1=lm3(tin["a_top_multi_direct"]))
                nc.gpsimd.tensor_mul(out=a1, in0=ff_sh,
                                     in1=lm3(tin["a_top_multi_diffuse"]))

                # fused pair reduction over (pair, c) -> per-layer sums
                u_red = u_pair.rearrange("p (t l c) -> p t l c", t=2, c=C
                                         ).transpose([0, 2, 1, 3])
                a_red = a_pair.rearrange("p (t l c) -> p t l c", t=2, c=C
                                         ).transpose([0, 2, 1, 3])
                nc.vector.tensor_reduce(out=st["t_ups"][:, l0 + 1:l0 + LB + 1],
                                        in_=u_red, axis=AX.XY, op=AL.add)
                nc.vector.tensor_reduce(out=st["t_abs"][:, l0:l0 + LB],
                                        in_=a_red, axis=AX.XY, op=AL.add)

                # per-level channel sums of the carries: ACT transposes
                # the carry-in slots to l-major lanes so the DVE reduce
                # gets a contiguous innermost axis (2x faster than
                # reducing the c-major grid directly). Covers levels
                # l0..l0+LB-1; the final level L is a small fixup below.
                fd_lane = pool_prod.tile([P, LB * C], f32, tag="fd_lane", bufs=2)
                ff_lane = pool_prod.tile([P, LB * C], f32, tag="ff_lane", bufs=2)
                nc.scalar.copy(out=lm3(fd_lane), in_=fd_sh)
                nc.scalar.copy(out=lm3(ff_lane), in_=ff_sh)
                nc.vector.tensor_reduce(out=st["t_fds"][:, l0:l0 + LB],
                                        in_=lm3(fd_lane), axis=AX.X,
                                        op=AL.add)
                nc.vector.tensor_reduce(out=st["t_ffs"][:, l0:l0 + LB],
                                        in_=lm3(ff_lane), axis=AX.X,
                                        op=AL.add)
                if bl == NBLK - 1:
                    nc.vector.tensor_reduce(
                        out=st["t_fds"][:, L:L + 1],
                        in_=fd_c3.transpose([0, 2, 1])[:, LB:LB + 1, :],
                        axis=AX.X, op=AL.add)
                    nc.vector.tensor_reduce(
                        out=st["t_ffs"][:, L:L + 1],
                        in_=ff_c3.transpose([0, 2, 1])[:, LB:LB + 1, :],
                        axis=AX.X, op=AL.add)

                st["prev_fd_c3"], st["prev_ff_c3"] = fd_c3, ff_c3

            def chunk_epilogue(st):
                r0 = st["r0"]
                nc.sync.dma_start(out=d_fds[r0:r0 + P], in_=st["t_fds"][:])
                nc.sync.dma_start(out=d_ffs[r0:r0 + P], in_=st["t_ffs"][:])
                nc.sync.dma_start(out=d_ups[r0:r0 + P], in_=st["t_ups"][:])
                nc.sync.dma_start(out=d_abs[r0:r0 + P], in_=st["t_abs"][:])

            # Software-pipeline two independent chunks at block granularity:
            # each engine's FIFO alternates between the two carry chains, so
            # while one chain waits on a scan the engine runs the other
            # chain's ready work instead of head-of-line stalling.
            for ch0 in range(0, n_chunks, 2):
                group = [chunk_prologue(ch0 + s)
                         for s in range(min(2, n_chunks - ch0))]
                for bl in range(NBLK):
                    for st in group:
                        chunk_block(st, bl)
                for st in group:
                    chunk_epilogue(st)

    nc.compile()
    return nc


_NC_CACHE = {}


def _get_nc(key=("full",)):
    if key not in _NC_CACHE:
        _NC_CACHE[key] = build_nc()
    return _NC_CACHE[key]


def kernel(**inputs):
    """Full-problem entry point: shard over 8 cores, run, gather."""
    from concourse.bass_utils import run_bass_kernel_spmd

    nc = _get_nc()
    rows = _B // _NCORES
    in_maps = []
    for core in range(_NCORES):
        sl = slice(core * rows, (core + 1) * rows)
        m = {n: np.ascontiguousarray(np.asarray(inputs[n])[sl], dtype=np.float32)
             for n in PROPS + FLUX}
        in_maps.append(m)

    res = run_bass_kernel_spmd(nc, in_maps, core_ids=list(range(_NCORES)))
    fds = np.concatenate([r["out_fds"] for r in res.results], axis=0)
    ffs = np.concatenate([r["out_ffs"] for r in res.results], axis=0)
    ups = np.concatenate([r["out_ups"] for r in res.results], axis=0)
    abs_ = np.concatenate([r["out_abs"] for r in res.results], axis=0)
    return fds, ffs, ups, abs_


# revision 12
# speedup vs baseline: 1.2669x; 1.0631x over previous
"""Trainium2 Bass kernel for nn_DownwardPropagation.

Math (per batch row b, channel c, layers l=1..L):
    fd_l = fd_{l-1} * td_l                       (direct downward flux)
    ff_l = (tdf_l + tmf_l) * ff_{l-1} + fd_{l-1} * tmd_l
    up_l = fd_{l-1} * rbd_l + ff_{l-1} * rbf_l
    ab_l = fd_{l-1} * atd_l + ff_{l-1} * atf_l
Outputs are channel sums: fd_sums[B, L+1], ff_sums[B, L+1],
up_sums[B, L+1] (up_0 = sum_c fd_0 * r_multi_direct), ab_sums[B, L].

Kernel layout: batch sharded 8 ways (2048 rows/core), each core processes
16 chunks of 128 rows (partition dim). Per chunk, layers are processed in
blocks of LB. The layer recurrences run as ONE tensor_tensor_scan per
carry per block over a c-major "grid" layout [C, G=LB+1]: slot (c,0) is a
boundary element with data0=0 (kills the cross-channel state leak) and
data1=carry-in (injects the per-channel initial value); slots (c,1..LB)
hold the per-layer coefficients. The scan output grid then holds
fd_{l0..l0+LB} per channel directly.
"""

import numpy as np

_B, _L, _C = 16384, 60, 48
_NCORES = 8
_P = 128
_LB = 20

PROPS = [
    "t_direct", "t_diffuse", "t_multi_direct", "t_multi_diffuse",
    "r_bottom_multi_direct", "r_bottom_multi_diffuse",
    "a_top_multi_direct", "a_top_multi_diffuse",
]
FLUX = ["r_multi_direct", "flux_down_above_direct", "flux_down_above_diffuse"]


def build_nc(n_rows=_B // _NCORES, L=_L, C=_C, LB=_LB, n_cores=_NCORES):
    import concourse.bacc as bacc
    import concourse.mybir as mybir
    from concourse.tile import TileContext

    f32 = mybir.dt.float32
    AL = mybir.AluOpType
    AX = mybir.AxisListType
    P = _P
    assert n_rows % P == 0 and L % LB == 0
    n_chunks = n_rows // P
    NBLK = L // LB
    G = LB + 1

    nc = bacc.Bacc("TRN2", target_bir_lowering=False, debug=False,
                   num_devices=n_cores)

    d_in = {n: nc.dram_tensor(n, [n_rows, L, C], f32, kind="ExternalInput").ap()
            for n in PROPS}
    d_fx = {n: nc.dram_tensor(n, [n_rows, C], f32, kind="ExternalInput").ap()
            for n in FLUX}
    d_fds = nc.dram_tensor("out_fds", [n_rows, L + 1], f32, kind="ExternalOutput").ap()
    d_ffs = nc.dram_tensor("out_ffs", [n_rows, L + 1], f32, kind="ExternalOutput").ap()
    d_ups = nc.dram_tensor("out_ups", [n_rows, L + 1], f32, kind="ExternalOutput").ap()
    d_abs = nc.dram_tensor("out_abs", [n_rows, L], f32, kind="ExternalOutput").ap()

    with TileContext(nc) as tc:
        with (
            tc.tile_pool(name="inp", bufs=2) as pool_in,
            tc.tile_pool(name="grids", bufs=2) as pool_grid,
            tc.tile_pool(name="seq", bufs=3) as pool_seq,
            tc.tile_pool(name="prod", bufs=3) as pool_prod,
            tc.tile_pool(name="outs", bufs=2) as pool_out,
            tc.tile_pool(name="small", bufs=2) as pool_small,
            tc.tile_pool(name="persist", bufs=1) as pool_persist,
        ):
            # Persistent injection grids, alternating per chunk parity.
            # Zeroed once; only their boundary column (tdc/ac: never) is
            # rewritten afterwards, so the zero interior persists.
            fd1 = [pool_persist.tile([P, C * G], f32, name=f"fd1_{i}", tag=f"fd1_{i}") for i in range(2)]
            tdc = [pool_persist.tile([P, C * G], f32, name=f"tdc_{i}", tag=f"tdc_{i}") for i in range(2)]
            ac = [pool_persist.tile([P, C * G], f32, name=f"ac_{i}", tag=f"ac_{i}") for i in range(2)]
            for t in (*fd1, *tdc, *ac):
                nc.vector.memset(t[:], 0.0)

            def co3(t):  # c-order view of an l-major [P, LB*C] tile -> [P, C, LB]
                return t.rearrange("p (l c) -> p l c", c=C).transpose([0, 2, 1])

            def lm3(t):  # l-major view [P, LB, C], contiguous
                return t.rearrange("p (l c) -> p l c", c=C)

            def chunk_prologue(ch):
                r0 = ch * P
                st = {"ch": ch, "r0": r0}
                st["fd1_3"] = fd1[ch % 2].rearrange("p (c g) -> p c g", g=G)
                st["tdc_g"], st["ac_g"], st["fd1_g"] = \
                    tdc[ch % 2], ac[ch % 2], fd1[ch % 2]
                st["tdc_3"] = tdc[ch % 2].rearrange("p (c g) -> p c g", g=G)
                st["ac_3"] = ac[ch % 2].rearrange("p (c g) -> p c g", g=G)

                fd0 = pool_small.tile([P, C], f32, tag="fd0")
                ff0 = pool_small.tile([P, C], f32, tag="ff0")
                rmd = pool_small.tile([P, C], f32, tag="rmd")
                nc.sync.dma_start(out=fd0[:], in_=d_fx["flux_down_above_direct"][r0:r0 + P])
                nc.sync.dma_start(out=ff0[:], in_=d_fx["flux_down_above_diffuse"][r0:r0 + P])
                nc.sync.dma_start(out=rmd[:], in_=d_fx["r_multi_direct"][r0:r0 + P])
                st["fd0"], st["ff0"] = fd0, ff0

                st["t_fds"] = pool_out.tile([P, L + 1], f32, name="t_fds", tag="o_fds")
                st["t_ffs"] = pool_out.tile([P, L + 1], f32, name="t_ffs", tag="o_ffs")
                st["t_ups"] = pool_out.tile([P, L + 1], f32, name="t_ups", tag="o_ups")
                st["t_abs"] = pool_out.tile([P, L], f32, name="t_abs", tag="o_abs")

                # up_0 = sum_c fd0 * rmd  (accum_out of a fused mult)
                trash = pool_small.tile([P, C], f32, tag="trash")
                nc.vector.scalar_tensor_tensor(
                    out=trash[:], in0=fd0[:], scalar=1.0, in1=rmd[:],
                    op0=AL.mult, op1=AL.mult, accum_out=st["t_ups"][:, 0:1])
                return st

            def chunk_block(st, bl):
                r0, l0 = st["r0"], bl * LB
                fd1_3, tdc_3, ac_3 = st["fd1_3"], st["tdc_3"], st["ac_3"]
                tin = {}
                for name in PROPS:
                    t = pool_in.tile([P, LB * C], f32, name=f"in_{name}",
                                     tag=f"in_{name}",
                                     bufs=3 if name in PROPS[:4] else 2)
                    nc.sync.dma_start(
                        out=t[:],
                        in_=d_in[name][r0:r0 + P, l0:l0 + LB].rearrange(
                            "p l c -> p (l c)"))
                    tin[name] = t

                # boundary injection for the fd scan (data1 col 0)
                if bl == 0:
                    nc.scalar.copy(out=fd1_3[:, :, 0:1],
                                   in_=st["fd0"][:].unsqueeze(2))
                else:
                    nc.scalar.copy(out=fd1_3[:, :, 0:1],
                                   in_=st["prev_fd_c3"][:, :, LB:LB + 1])
                # td -> c-major grid (cols 1..G); col 0 stays 0
                nc.scalar.copy(out=tdc_3[:, :, 1:G], in_=co3(tin["t_direct"]))
                # a = tdf + tmf: contiguous add on DVE, then ACT moves it
                # into the c-major grid (strided APs are ~2x slower on
                # DVE but cheap to absorb on the ACT engine)
                a_lm = pool_grid.tile([P, LB * C], f32, tag="a_lm")
                nc.vector.tensor_add(out=a_lm[:], in0=tin["t_diffuse"][:],
                                     in1=tin["t_multi_diffuse"][:])
                nc.scalar.copy(out=ac_3[:, :, 1:G], in_=co3(a_lm))
                # tmd -> c-major grid so the b-mult below is all-contiguous
                tmd_c = pool_grid.tile([P, C * G], f32, tag="tmd_c")
                tmdc_3 = tmd_c.rearrange("p (c g) -> p c g", g=G)
                nc.scalar.copy(out=tmdc_3[:, :, 1:G],
                               in_=co3(tin["t_multi_direct"]))

                fd_c = pool_seq.tile([P, C * G], f32, tag="fd_c")
                nc.vector.tensor_tensor_scan(
                    out=fd_c[:], data0=st["tdc_g"][:], data1=st["fd1_g"][:],
                    initial=0.0, op0=AL.mult, op1=AL.add)
                fd_c3 = fd_c.rearrange("p (c g) -> p c g", g=G)

                # b grid: col0 = ff carry, cols 1.. = fd_{l-1} * tmd_l
                b_buf = pool_grid.tile([P, C * G], f32, tag="b_buf", bufs=2)
                b_3 = b_buf.rearrange("p (c g) -> p c g", g=G)
                if bl == 0:
                    nc.scalar.copy(out=b_3[:, :, 0:1],
                                   in_=st["ff0"][:].unsqueeze(2))
                else:
                    nc.scalar.copy(out=b_3[:, :, 0:1],
                                   in_=st["prev_ff_c3"][:, :, LB:LB + 1])
                nc.gpsimd.tensor_mul(out=b_3[:, :, 1:G],
                                     in0=fd_c3[:, :, 0:LB],
                                     in1=tmdc_3[:, :, 1:G])

                ff_c = pool_seq.tile([P, C * G], f32, tag="ff_c")
                nc.vector.tensor_tensor_scan(
                    out=ff_c[:], data0=st["ac_g"][:], data1=b_buf[:],
                    initial=0.0, op0=AL.mult, op1=AL.add)
                ff_c3 = ff_c.rearrange("p (c g) -> p c g", g=G)

                # shifted (carry-in) sequences, l-major views [P, LB, C]
                fd_sh = fd_c3.transpose([0, 2, 1])[:, 0:LB, :]
                ff_sh = ff_c3.transpose([0, 2, 1])[:, 0:LB, :]

                # ACT transposes the carry-in slots to l-major lanes FIRST:
                # the per-level reduces AND all four products then run on
                # fully contiguous APs (1 cyc/elem on DVE instead of the
                # ~2x strided penalty / ~3x on GPSIMD).
                fd_lane = pool_prod.tile([P, LB * C], f32, tag="fd_lane", bufs=2)
                ff_lane = pool_prod.tile([P, LB * C], f32, tag="ff_lane", bufs=2)
                nc.scalar.copy(out=lm3(fd_lane), in_=fd_sh)
                nc.scalar.copy(out=lm3(ff_lane), in_=ff_sh)

                # products, contiguous; split DVE/GPSIMD for balance
                u_pair = pool_prod.tile([P, 2 * LB * C], f32, tag="u_pair", bufs=2)
                a_pair = pool_prod.tile([P, 2 * LB * C], f32, tag="a_pair", bufs=2)
                u0 = u_pair[:, 0:LB * C]
                u1 = u_pair[:, LB * C:]
                a0 = a_pair[:, 0:LB * C]
                a1 = a_pair[:, LB * C:]
                nc.gpsimd.tensor_mul(out=u0, in0=fd_lane[:],
                                     in1=tin["r_bottom_multi_direct"][:])
                nc.gpsimd.tensor_mul(out=u1, in0=ff_lane[:],
                                     in1=tin["r_bottom_multi_diffuse"][:])
                nc.vector.tensor_mul(out=a0, in0=fd_lane[:],
                                     in1=tin["a_top_multi_direct"][:])
                nc.gpsimd.tensor_mul(out=a1, in0=ff_lane[:],
                                     in1=tin["a_top_multi_diffuse"][:])

                # fused pair reduction over (pair, c) -> per-layer sums
                u_red = u_pair.rearrange("p (t l c) -> p t l c", t=2, c=C
                                         ).transpose([0, 2, 1, 3])
                a_red = a_pair.rearrange("p (t l c) -> p t l c", t=2, c=C
                                         ).transpose([0, 2, 1, 3])
                nc.vector.tensor_reduce(out=st["t_ups"][:, l0 + 1:l0 + LB + 1],
                                        in_=u_red, axis=AX.XY, op=AL.add)
                nc.vector.tensor_reduce(out=st["t_abs"][:, l0:l0 + LB],
                                        in_=a_red, axis=AX.XY, op=AL.add)

                # per-level channel sums of the carries (contiguous lanes)
                nc.vector.tensor_reduce(out=st["t_fds"][:, l0:l0 + LB],
                                        in_=lm3(fd_lane), axis=AX.X,
                                        op=AL.add)
                nc.vector.tensor_reduce(out=st["t_ffs"][:, l0:l0 + LB],
                                        in_=lm3(ff_lane), axis=AX.X,
                                        op=AL.add)
                if bl == NBLK - 1:
                    nc.vector.tensor_reduce(
                        out=st["t_fds"][:, L:L + 1],
                        in_=fd_c3.transpose([0, 2, 1])[:, LB:LB + 1, :],
                        axis=AX.X, op=AL.add)
                    nc.vector.tensor_reduce(
                        out=st["t_ffs"][:, L:L + 1],
                        in_=ff_c3.transpose([0, 2, 1])[:, LB:LB + 1, :],
                        axis=AX.X, op=AL.add)

                st["prev_fd_c3"], st["prev_ff_c3"] = fd_c3, ff_c3

            def chunk_epilogue(st):
                r0 = st["r0"]
                nc.sync.dma_start(out=d_fds[r0:r0 + P], in_=st["t_fds"][:])
                nc.sync.dma_start(out=d_ffs[r0:r0 + P], in_=st["t_ffs"][:])
                nc.sync.dma_start(out=d_ups[r0:r0 + P], in_=st["t_ups"][:])
                nc.sync.dma_start(out=d_abs[r0:r0 + P], in_=st["t_abs"][:])

            # Software-pipeline two independent chunks at block granularity:
            # each engine's FIFO alternates between the two carry chains, so
            # while one chain waits on a scan the engine runs the other
            # chain's ready work instead of head-of-line stalling.
            for ch0 in range(0, n_chunks, 2):
                group = [chunk_prologue(ch0 + s)
                         for s in range(min(2, n_chunks - ch0))]
                for bl in range(NBLK):
                    for st in group:
                        chunk_block(st, bl)
                for st in group:
                    chunk_epilogue(st)

    nc.compile()
    return nc


_NC_CACHE = {}


def _get_nc(key=("full",)):
    if key not in _NC_CACHE:
        _NC_CACHE[key] = build_nc()
    return _NC_CACHE[key]


def kernel(**inputs):
    """Full-problem entry point: shard over 8 cores, run, gather."""
    from concourse.bass_utils import run_bass_kernel_spmd

    nc = _get_nc()
    rows = _B // _NCORES
    in_maps = []
    for core in range(_NCORES):
        sl = slice(core * rows, (core + 1) * rows)
        m = {n: np.ascontiguousarray(np.asarray(inputs[n])[sl], dtype=np.float32)
             for n in PROPS + FLUX}
        in_maps.append(m)

    res = run_bass_kernel_spmd(nc, in_maps, core_ids=list(range(_NCORES)))
    fds = np.concatenate([r["out_fds"] for r in res.results], axis=0)
    ffs = np.concatenate([r["out_ffs"] for r in res.results], axis=0)
    ups = np.concatenate([r["out_ups"] for r in res.results], axis=0)
    abs_ = np.concatenate([r["out_abs"] for r in res.results], axis=0)
    return fds, ffs, ups, abs_
